# revision 34
# baseline (speedup 1.0000x reference)
"""Trainium2 SPMD kernel: StyleGAN2-style modulated conv (Conv2dWeightModulate).

Reference math (per batch sample b):
    w0        = weight * RC                       (equalized-lr scale)
    ws        = w0 * style[b][None,:,None,None]   (per-input-channel modulation)
    sigma_inv = rsqrt(sum_{I,K,K} ws^2 + eps)     (per-output-channel demodulation)
    out[b]    = conv2d(x[b], ws * sigma_inv, pad=1)

Because the modulation is a per-input-channel scale and conv is linear, this
factorizes into ops with a SHARED weight across the batch:
    out[b] = sigma_inv[b,:] * conv2d(x[b] * (style[b]*RC), weight)
    sigma_inv[b,o] = rsqrt(RC^2 * sum_{i,t} weight[o,i,t]^2 * style[b,i]^2 + eps)

Sharding: data-parallel over batch: 8 samples -> 8 NeuronCores, weight
replicated (the groups=b conv factorizes exactly across the batch).

Schedule (v9, ~93us; hardware model from ntff trace analysis, v1 @96.3us):
  - Fixed costs: ~6.9us framework preamble, ~2.7us teardown after last DMA.
  - DMA facts: sems post per-DMA but only when the SLOWEST of the 16 shared
    engines drains (cross-queue contention => stragglers, +2us seen); a
    drained queue pays ~1.4-2us issue->first-packet, but chains seamlessly
    while busy; a ring holds ~4 outstanding DMAs - further issues BLOCK the
    issuing engine (capacity wait); Q1(sync) wakes ~1.5-2us, Q10(scalar)
    ~2.5-3us.
  - sync ring (Q1): ONE fused DMA [128, 4+1024] = host-TRANSPOSED style
    [128,4] (pure permutation) + x0 - lands together ~10.5, killing the
    old style->PE-transpose->st chain. x1-3 gated behind x-scale
    consumption (tensor_copy reading xs -> WAR on xst123) so their packets
    don't contend with Q10's critical head weight slices.
  - scalar ring (Q10): 13 ungated weight slices, fine at the head
    ([0:1],[1:3],[3:5],[5:7],[7:9], then 3-tap) so tap delivery beats PE
    consumption (1 tap/1.7us at half clock, 1/us at full).
  - PE: ~20 tiny warmups (pipeline warm only - HAM's full-clock grant
    follows the first HIGH-TOGGLE matmul by ~4.3us; constant-data primers
    of any size do NOT move it, so ~2.2us of half-clock conv is
    unavoidable); conv stream gap-free at ~249ns/512-col fp32r matmul
    (fp32r = 1.0 cycles/row at >=256 cols, same as bf16 - bf16 buys
    nothing; fp8 DoubleRow would be 2x but e4m3 noise ~2.5e-2 > gate).
  - sigma: squares on SCALAR (queue behind the capacity-paced dma issues,
    done ~58; NOT DVE - the tile scheduler hoists weight-gated squares
    ahead of the critical style/x chain; NOT gpsimd - much slower AND its
    load triggers HAM duty-cycling, +30us), DVE tap-sums, PE reduction vs
    style^2 at wave-B g5 chunk boundaries, sqrt -> PE-transpose ->
    reciprocal.
  - waves: A = 4 oc-groups x h=0 interleaved (tap-major); B = per-group
    serial chains; wave-A flushes + h0 outs after sigma (~68); the final
    h1 pair stored as per-oc DMAs so only 256KB trails the last matmul
    (tail ~2.9us incl. one unavoidable ring wake).
"""

from contextlib import ExitStack

import numpy as np

import concourse.bass as bass
import concourse.tile as tile
from concourse import bacc, mybir
from concourse.bass_utils import run_bass_kernel_spmd

B = 8
CIN = 512
COUT = 512
KK = 3
H = 32
W = 32
PIX = H * W
NCH = 4  # channel chunks of 128
TAPS = KK * KK
RC = float(1.0 / np.sqrt(CIN * KK * KK))
EPS = 1e-8
F32 = mybir.dt.float32
F32R = mybir.dt.float32r
BF16 = mybir.dt.bfloat16
AF = mybir.ActivationFunctionType

# test.py toggles these; the grading harness just calls kernel().
TRACE = False
LAST_RESULTS = None

WARM_PRE = 16  # tiny warmups before style transposes (keep PE pipeline warm)
WARM_POST = 4  # tiny warmups between transposes and conv deps (-> ~11.3)


def _body(ctx, tc, x0s_d, x123_d, wt_d, out_d):
    nc = tc.nc
    const = ctx.enter_context(tc.tile_pool(name="const", bufs=1))
    wpool = ctx.enter_context(tc.tile_pool(name="wpool", bufs=1))
    xpool = ctx.enter_context(tc.tile_pool(name="xpool", bufs=1))
    sqpool = ctx.enter_context(tc.tile_pool(name="sqpool", bufs=3))
    opool = ctx.enter_context(tc.tile_pool(name="opool", bufs=1))
    psum = ctx.enter_context(
        tc.tile_pool(name="psum", bufs=1, space=bass.MemorySpace.PSUM)
    )
    sigpsum = ctx.enter_context(
        tc.tile_pool(name="sigpsum", bufs=1, space=bass.MemorySpace.PSUM)
    )

    # --- tiles ---
    st_rc = const.tile([128, NCH], F32, tag="st_rc")
    st2 = const.tile([128, NCH], BF16, tag="st2")

    wt = [
        wpool.tile([128, TAPS, COUT], F32R, tag=f"wt{c}", name=f"wt{c}")
        for c in range(NCH)
    ]
    xs = []
    for c in range(NCH):
        xc = xpool.tile([128, H + 2, W + 2], F32R, tag=f"xs{c}", name=f"xs{c}")
        xs.append(xc)
    # fused input: cols 0:4 = host-transposed style [128,4], cols 4: = x0
    xst0s = xpool.tile([128, NCH + PIX], F32, tag="xst0s", name="xst0s")
    xst123 = xpool.tile([128, NCH - 1, PIX], F32, tag="xst123", name="xst123")

    # tiny warmup operands (the HAM full-clock grant only follows REAL
    # high-toggle matmul activity ~4.3us later; constant-data primers of any
    # size don't move it, so warmups here only keep the PE pipeline warm)
    warm_src = const.tile([1, 128], F32R, tag="warm_src")
    ones_r = const.tile([1, 1], F32R, tag="ones_r")

    # one PSUM bank shared by: warmup dst + sigma accumulation row + sigma
    # transposes
    sigbank = sigpsum.tile([128, 512], F32, tag="sigbank")
    sig_ps = sigbank[0:1, :]
    warm_ps = sigbank[0:1, 0:128]

    eps_b = const.tile([1, 1], F32, tag="eps_b")
    ones_t = const.tile([1, 1], F32, tag="ones_t")
    sqrt_dummy = const.tile([1, 1], F32, tag="sqrt_dummy")
    sig_sq = const.tile([1, COUT], F32, tag="sig_sq")
    sig_sd = const.tile([128, NCH], F32, tag="sig_sd")
    sig_t = const.tile([128, NCH], F32, tag="sig_t")
    # per-h output staging: oc writes cols [oc*512:(oc+1)*512]
    ob_h = [
        opool.tile([128, NCH * 512], F32, tag=f"ob{h}", name=f"ob{h}")
        for h in range(2)
    ]

    # --- early memsets (no data deps) ---
    nc.gpsimd.memset(warm_src[:].bitcast(F32), 1.0)
    nc.gpsimd.memset(ones_r[:].bitcast(F32), 1.0)
    nc.vector.memset(eps_b[:], EPS)
    nc.vector.memset(ones_t[:], 1.0)
    nc.vector.memset(sqrt_dummy[:], 1.0)
    for c in range(NCH):
        eng = nc.vector if c == 0 else nc.gpsimd
        v = xs[c][:].bitcast(F32)
        eng.memset(v[:, 0, :], 0.0)
        eng.memset(v[:, H + 1, :], 0.0)
        eng.memset(v[:, 1 : H + 1, 0], 0.0)
        eng.memset(v[:, 1 : H + 1, W + 1], 0.0)

    # --- sync ring (Q1): ONE fused style+x0 DMA (consecutive small DMAs on
    # a drained queue each pay ~1.4us descriptor-refetch, so fusing beats
    # style-then-x0; sem ~9.9 covers both). x1-3 are gated behind x scales
    # (emitted later) so their packets don't contend with Q10's critical
    # head weight slices on the shared DMA engines (a DMA's sem posts when
    # its SLOWEST engine drains - contention = stragglers). ---
    nc.sync.dma_start(xst0s[:], x0s_d[:])

    # --- scalar ring (Q10): weight slices back-to-back, ungated. Head
    # slices sized so tap delivery beats PE consumption: conv eats 1 tap per
    # ~1.7us at half clock from ~11.4, 1/us after the HAM grant (~conv+4.3).
    # The scalar engine capacity-blocks when >4 DMAs are outstanding on the
    # ring, so it carries ONLY these issues (squares live on DVE). ---
    nc.scalar.dma_start(wt[0][:, 0:1], wt_d[:, 0, 0:1])
    nc.scalar.dma_start(wt[0][:, 1:3], wt_d[:, 0, 1:3])
    nc.scalar.dma_start(wt[0][:, 3:5], wt_d[:, 0, 3:5])
    nc.scalar.dma_start(wt[0][:, 5:7], wt_d[:, 0, 5:7])
    nc.scalar.dma_start(wt[0][:, 7:9], wt_d[:, 0, 7:9])
    for c in range(1, NCH):
        nc.scalar.dma_start(wt[c][:, 0:3], wt_d[:, c, 0:3])
        nc.scalar.dma_start(wt[c][:, 3:6], wt_d[:, c, 3:6])
        nc.scalar.dma_start(wt[c][:, 6:9], wt_d[:, c, 6:9])
    # prefetch the Sqrt ACT table (1.3us load) off the critical path
    nc.scalar.activation(sqrt_dummy[:], sqrt_dummy[:], AF.Sqrt)

    # --- sigma squares on SCALAR (ACT Square; they queue behind the
    # capacity-paced dma issues and run ~34us on - fine, sigma is consumed
    # at wave-B g5). NOT on DVE (the tile scheduler interleaves weight-gated
    # squares ahead of the critical style/x-scale chain) and NOT on gpsimd
    # (gpsimd tensor ops are far slower and the load triggers HAM
    # duty-cycling). sqpool's 3-buf rotation paces against the DVE sums. ---
    w2tiles = {}

    def sq(c, t):
        w2 = sqpool.tile([128, COUT], BF16, tag=f"w2_{t % 3}", name="w2")
        nc.scalar.activation(w2[:], wt[c][:, t], AF.Square)
        w2tiles[(c, t)] = w2

    # --- PE: tiny warmups (no deps -> run from ~6.9us) keep the pipeline
    # warm until conv deps land (~11.3) ---
    for _ in range(WARM_PRE + WARM_POST):
        nc.tensor.matmul(warm_ps, ones_r[:], warm_src[:], start=True, stop=True)

    # --- style scales (DVE; straight off the fused tile's first 4 cols) ---
    stv = xst0s[:, 0:NCH]
    nc.vector.tensor_scalar_mul(st_rc[:], stv, RC)
    nc.vector.tensor_mul(st2[:], stv, stv)

    # --- x modulation (DVE); x0 scaled in halves so conv's first taps can
    # start right after the fused DMA lands ---
    x0v = xst0s[:, NCH:].rearrange("p (h w) -> p h w", h=H)
    nc.vector.tensor_scalar_mul(
        xs[0][:, 1:17, 1 : W + 1], x0v[:, 0:16, :], st_rc[:, 0:1]
    )
    nc.vector.tensor_scalar_mul(
        xs[0][:, 17 : H + 1, 1 : W + 1], x0v[:, 16:32, :], st_rc[:, 0:1]
    )
    # gate: this copy READS scaled xs[0] and WRITES a corner of xst123's
    # chunk-1 region, so the x1 DMA below (WAR) can't start its packets
    # until x0 is scaled - keeping Q1 quiet during Q10's critical head
    # slices. x2/x3 are likewise gated behind xs1's scale.
    nc.vector.tensor_copy(xst123[0:1, 0, 0:1], xs[0][0:1, 1, 1:2].bitcast(F32))
    nc.sync.dma_start(xst123[:, 0], x123_d[0])
    nc.vector.tensor_scalar_mul(
        xs[1][:, 1 : H + 1, 1 : W + 1],
        xst123[:, 0].rearrange("p (h w) -> p h w", h=H),
        st_rc[:, 1:2],
    )
    nc.vector.tensor_copy(xst123[0:1, 1, 0:1], xs[1][0:1, 1, 1:2].bitcast(F32))
    nc.sync.dma_start(xst123[:, 1], x123_d[1])
    nc.sync.dma_start(xst123[:, 2], x123_d[2])
    for c in range(2, NCH):
        nc.vector.tensor_scalar_mul(
            xs[c][:, 1 : H + 1, 1 : W + 1],
            xst123[:, c - 1].rearrange("p (h w) -> p h w", h=H),
            st_rc[:, c : c + 1],
        )

    # --- scalar squares + DVE tap-sums for sigma, interleaved so the 3-buf
    # w2 rotation never overwrites an unconsumed square ---
    w2s = {}

    def sq_adds(c):
        acc = sqpool.tile([128, COUT], BF16, tag=f"w2s{c}", name="w2s")
        sq(c, 0)
        sq(c, 1)
        nc.vector.tensor_add(acc[:], w2tiles[(c, 0)][:], w2tiles[(c, 1)][:])
        for t in range(2, TAPS):
            sq(c, t)
            nc.vector.tensor_add(acc[:], acc[:], w2tiles[(c, t)][:])
        w2s[c] = acc

    for c in range(NCH):
        sq_adds(c)

    # wave A groups: (oc, h=0) -> banks pc0-3; wave B: g4-6 fresh banks, g7
    # reuses pc0 (freed by wave A's flush long before g7 starts)
    wave_a = [(oc, 0) for oc in range(NCH)]
    wave_b = [(oc, 1) for oc in range(NCH)]
    pc = {
        g: psum.tile([128, 512], F32, tag=f"pc{i}", name=f"pc{i}")
        for i, g in enumerate(wave_a)
    }

    def conv_mm(g, c, t, start, stop):
        oc, h = g
        dy, dx = t // 3, t % 3
        h0 = h * 16
        nc.tensor.matmul(
            pc[g][:],
            wt[c][:, t, oc * 128 : (oc + 1) * 128],
            xs[c][:, dy + h0 : dy + h0 + 16, dx : dx + W],
            start=start,
            stop=stop,
        )

    def sig_mm(c):
        nc.tensor.matmul(
            sig_ps, st2[:, c : c + 1], w2s[c][:], start=(c == 0), stop=(c == NCH - 1)
        )

    def sig_finalize():
        # sqrt(RC^2*q + eps) [1,512] -> PE-transpose -> [128,4] -> reciprocal
        # (an SBUF->SBUF DMA reshape has the wrong element order: the natural
        # [1,512]->[128,4] mapping is partition-major, the flush needs
        # column-major)
        nc.scalar.activation(
            sig_sq[:], sig_ps, AF.Sqrt, bias=eps_b[:], scale=RC * RC
        )
        for oc in range(NCH):
            nc.tensor.transpose(
                sigbank[:, oc : oc + 1],
                sig_sq[0:1, oc * 128 : (oc + 1) * 128],
                ones_t[:],
            )
        nc.vector.tensor_copy(sig_sd[:], sigbank[:, 0:NCH])
        nc.vector.reciprocal(sig_t[:], sig_sd[:])

    def flush(g):
        # PSUM -> SBUF scaled by sigma_inv, on DVE; waits on sig_t which lands
        # mid-wave-B - only g7's bank reuse (~26us later) depends on it.
        oc, h = g
        nc.vector.tensor_scalar_mul(
            ob_h[h][:, oc * 512 : (oc + 1) * 512], pc[g][:], sig_t[:, oc : oc + 1]
        )

    def out_dma_pair(h, pair):
        nc.sync.dma_start(
            out_d[h, pair, :, :],
            ob_h[h][:, pair * 1024 : (pair + 1) * 1024].rearrange(
                "p (a q) -> p a q", a=2
            ),
        )

    def out_dma_oc(h, oc):
        nc.sync.dma_start(
            out_d[h, oc // 2, :, oc % 2],
            ob_h[h][:, oc * 512 : (oc + 1) * 512],
        )

    # --- wave A (h=0): tap-major, group-minor ---
    for c in range(NCH):
        for t in range(TAPS):
            for g in wave_a:
                conv_mm(g, c, t, c == 0 and t == 0, c == NCH - 1 and t == TAPS - 1)

    # --- wave B (h=1): per-group serial chains; sigma reduction at g5 chunk
    # boundaries (the scalar squares only finish ~34us on, after the
    # capacity-paced dma issues), finalize after g5. All flush emissions
    # come after sig_finalize so sig_t's write precedes every flush in DVE
    # program order. ---
    for gi, g in enumerate(wave_b):
        tag = "pc0" if gi == 3 else f"pcB{gi}"
        pc[g] = psum.tile([128, 512], F32, tag=tag, name=f"pcb{gi}")
        k = 0
        for c in range(NCH):
            if gi == 1:
                sig_mm(c)
            for t in range(TAPS):
                conv_mm(g, c, t, k == 0, k == TAPS * NCH - 1)
                k += 1
        if gi == 1:
            sig_finalize()
            for ga in wave_a:
                flush(ga)
            out_dma_pair(0, 0)
            out_dma_pair(0, 1)
            flush(wave_b[0])
            flush(g)
        elif gi == 2:
            flush(g)
            out_dma_pair(1, 0)
            out_dma_oc(1, 2)
        elif gi == 3:
            flush(g)
            out_dma_oc(1, 3)


_CACHE = None


def _get_compiled():
    global _CACHE
    if _CACHE is None:
        nc = bacc.Bacc(
            "TRN2", target_bir_lowering=False, debug=False, num_devices=B
        )
        # x0s: cols 0:4 = host-transposed style [128,4], cols 4: = x chunk 0
        x0s_d = nc.dram_tensor(
            "x0s", [128, NCH + PIX], F32, kind="ExternalInput"
        ).ap()
        x123_d = nc.dram_tensor(
            "x123", [NCH - 1, 128, PIX], F32, kind="ExternalInput"
        ).ap()
        wt_d = nc.dram_tensor(
            "wt", [128, NCH, TAPS, COUT], F32R, kind="ExternalInput"
        ).ap()
        # out layout: [h, oc_pair, 128, 2, 512] -> host reassembles
        out_d = nc.dram_tensor(
            "out", [2, 2, 128, 2, 512], F32, kind="ExternalOutput"
        ).ap()
        with tile.TileContext(nc) as tc, ExitStack() as ctx:
            _body(ctx, tc, x0s_d, x123_d, wt_d, out_d)
        nc.compile()
        _CACHE = nc
    return _CACHE


def kernel(x, style, weight):
    """x: (8,512,32,32) f32, style: (8,512) f32, weight: (512,512,3,3) f32
    -> (8,512,32,32) f32"""
    global LAST_RESULTS
    x = np.ascontiguousarray(np.asarray(x, dtype=np.float32))
    style = np.asarray(style, dtype=np.float32)
    weight = np.asarray(weight, dtype=np.float32)

    # Host-side layout only (no arithmetic): lhsT weight layout
    # wt[i_lo, c, t, o] = weight[o, c*128 + i_lo, t//3, t%3]
    wt = np.ascontiguousarray(
        weight.reshape(COUT, NCH, 128, TAPS).transpose(2, 1, 3, 0)
    )
    in_maps = []
    for b in range(B):
        xb = x[b].reshape(NCH, 128, PIX)
        # fused tile: transposed style [128,4] (st[p,c]=style[c*128+p],
        # a pure permutation) next to x chunk 0
        x0s = np.ascontiguousarray(
            np.concatenate([style[b].reshape(NCH, 128).T, xb[0]], axis=1)
        )
        in_maps.append(
            {
                "x0s": x0s,
                "x123": np.ascontiguousarray(xb[1:]),
                "wt": wt,
            }
        )

    nc = _get_compiled()
    res = run_bass_kernel_spmd(nc, in_maps, list(range(B)), trace=TRACE)
    LAST_RESULTS = res
    out = np.empty((B, COUT, H, W), dtype=np.float32)
    for b in range(B):
        # out HBM [h, pair, 128(cout_lo), j(oc in pair), 512(pix half)]
        o = res.results[b]["out"]
        o = o.transpose(1, 3, 2, 0, 4)  # -> [pair, j, cout_lo, h, q]
        out[b] = o.reshape(COUT, H, W)
    return out


# revision 39
# speedup vs baseline: 1.0139x; 1.0139x over previous
"""Trainium2 SPMD kernel: StyleGAN2-style modulated conv (Conv2dWeightModulate).

Reference math (per batch sample b):
    w0        = weight * RC                       (equalized-lr scale)
    ws        = w0 * style[b][None,:,None,None]   (per-input-channel modulation)
    sigma_inv = rsqrt(sum_{I,K,K} ws^2 + eps)     (per-output-channel demodulation)
    out[b]    = conv2d(x[b], ws * sigma_inv, pad=1)

Because the modulation is a per-input-channel scale and conv is linear, this
factorizes into ops with a SHARED weight across the batch:
    out[b] = sigma_inv[b,:] * conv2d(x[b] * (style[b]*RC), weight)
    sigma_inv[b,o] = rsqrt(RC^2 * sum_{i,t} weight[o,i,t]^2 * style[b,i]^2 + eps)

Sharding: data-parallel over batch: 8 samples -> 8 NeuronCores, weight
replicated (the groups=b conv factorizes exactly across the batch).

Schedule (v9, ~93us; hardware model from ntff trace analysis, v1 @96.3us):
  - Fixed costs: ~6.9us framework preamble, ~2.7us teardown after last DMA.
  - DMA facts: sems post per-DMA but only when the SLOWEST of the 16 shared
    engines drains (cross-queue contention => stragglers, +2us seen); a
    drained queue pays ~1.4-2us issue->first-packet, but chains seamlessly
    while busy; a ring holds ~4 outstanding DMAs - further issues BLOCK the
    issuing engine (capacity wait); Q1(sync) wakes ~1.5-2us, Q10(scalar)
    ~2.5-3us.
  - sync ring (Q1): ONE fused DMA [128, 4+1024] = host-TRANSPOSED style
    [128,4] (pure permutation) + x0 - lands together ~10.5, killing the
    old style->PE-transpose->st chain. x1-3 gated behind x-scale
    consumption (tensor_copy reading xs -> WAR on xst123) so their packets
    don't contend with Q10's critical head weight slices.
  - scalar ring (Q10): 13 ungated weight slices, fine at the head
    ([0:1],[1:3],[3:5],[5:7],[7:9], then 3-tap) so tap delivery beats PE
    consumption (1 tap/1.7us at half clock, 1/us at full).
  - PE: ~20 tiny warmups (pipeline warm only - HAM's full-clock grant
    follows the first HIGH-TOGGLE matmul by ~4.3us; constant-data primers
    of any size do NOT move it, so ~2.2us of half-clock conv is
    unavoidable); conv stream gap-free at ~249ns/512-col fp32r matmul
    (fp32r = 1.0 cycles/row at >=256 cols, same as bf16 - bf16 buys
    nothing; fp8 DoubleRow would be 2x but e4m3 noise ~2.5e-2 > gate).
  - sigma: squares on SCALAR (queue behind the capacity-paced dma issues,
    done ~58; NOT DVE - the tile scheduler hoists weight-gated squares
    ahead of the critical style/x chain; NOT gpsimd - much slower AND its
    load triggers HAM duty-cycling, +30us), DVE tap-sums, PE reduction vs
    style^2 at wave-B g5 chunk boundaries, sqrt -> PE-transpose ->
    reciprocal.
  - waves: A = 4 oc-groups x h=0 interleaved (tap-major); B = per-group
    serial chains; wave-A flushes + h0 outs after sigma (~68); the final
    h1 pair stored as per-oc DMAs so only 256KB trails the last matmul
    (tail ~2.9us incl. one unavoidable ring wake).
"""

from contextlib import ExitStack

import numpy as np

import concourse.bass as bass
import concourse.tile as tile
from concourse import bacc, mybir
from concourse.bass_utils import run_bass_kernel_spmd

B = 8
CIN = 512
COUT = 512
KK = 3
H = 32
W = 32
PIX = H * W
NCH = 4  # channel chunks of 128
TAPS = KK * KK
RC = float(1.0 / np.sqrt(CIN * KK * KK))
EPS = 1e-8
F32 = mybir.dt.float32
F32R = mybir.dt.float32r
BF16 = mybir.dt.bfloat16
AF = mybir.ActivationFunctionType

# test.py toggles these; the grading harness just calls kernel().
TRACE = False
LAST_RESULTS = None

WARM_BIG = 9  # random-data conv-sized primers bridging ~7.2 -> ~11.3us


def _body(ctx, tc, x0s_d, x123_d, wt_d, out_d):
    nc = tc.nc
    const = ctx.enter_context(tc.tile_pool(name="const", bufs=1))
    wpool = ctx.enter_context(tc.tile_pool(name="wpool", bufs=1))
    xpool = ctx.enter_context(tc.tile_pool(name="xpool", bufs=1))
    sqpool = ctx.enter_context(tc.tile_pool(name="sqpool", bufs=3))
    opool = ctx.enter_context(tc.tile_pool(name="opool", bufs=1))
    psum = ctx.enter_context(
        tc.tile_pool(name="psum", bufs=1, space=bass.MemorySpace.PSUM)
    )
    sigpsum = ctx.enter_context(
        tc.tile_pool(name="sigpsum", bufs=1, space=bass.MemorySpace.PSUM)
    )

    # --- tiles ---
    st_rc = const.tile([128, NCH], F32, tag="st_rc")
    st2 = const.tile([128, NCH], BF16, tag="st2")

    wt = [
        wpool.tile([128, TAPS, COUT], F32R, tag=f"wt{c}", name=f"wt{c}")
        for c in range(NCH)
    ]
    xs = []
    for c in range(NCH):
        xc = xpool.tile([128, H + 2, W + 2], F32R, tag=f"xs{c}", name=f"xs{c}")
        xs.append(xc)
    # fused input: cols 0:4 = host-transposed style [128,4], cols 4: = x0
    xst0s = xpool.tile([128, NCH + PIX], F32, tag="xst0s", name="xst0s")
    xst123 = xpool.tile([128, NCH - 1, PIX], F32, tag="xst123", name="xst123")

    # warmup operands: RANDOM-data conv-sized primers (the HAM full-clock
    # grant follows high-TOGGLE matmul activity by ~4.3us; constant-data
    # primers of any size don't move it - hw-RNG fills give real toggle so
    # the grant can land before the conv stream starts)
    warm_src = const.tile([1, 128], F32R, tag="warm_src")
    ones_r = const.tile([1, 1], F32R, tag="ones_r")
    warm_lhs = const.tile([128, 128], F32R, tag="warm_lhs")
    warm_rhs = const.tile([128, 512], F32R, tag="warm_rhs")

    # one PSUM bank shared by: warmup dst + sigma accumulation row + sigma
    # transposes
    sigbank = sigpsum.tile([128, 512], F32, tag="sigbank")
    sig_ps = sigbank[0:1, :]
    warm_ps = sigbank[0:1, 0:128]

    eps_b = const.tile([1, 1], F32, tag="eps_b")
    ones_t = const.tile([1, 1], F32, tag="ones_t")
    sqrt_dummy = const.tile([1, 1], F32, tag="sqrt_dummy")
    sig_sq = const.tile([1, COUT], F32, tag="sig_sq")
    sig_sd = const.tile([128, NCH], F32, tag="sig_sd")
    sig_t = const.tile([128, NCH], F32, tag="sig_t")
    # per-h output staging: oc writes cols [oc*512:(oc+1)*512]
    ob_h = [
        opool.tile([128, NCH * 512], F32, tag=f"ob{h}", name=f"ob{h}")
        for h in range(2)
    ]

    # --- early memsets (no data deps) ---
    nc.gpsimd.memset(warm_src[:].bitcast(F32), 1.0)
    nc.gpsimd.memset(ones_r[:].bitcast(F32), 1.0)
    nc.gpsimd.random(warm_lhs[:])
    nc.gpsimd.random(warm_rhs[:])
    nc.vector.memset(eps_b[:], EPS)
    nc.vector.memset(ones_t[:], 1.0)
    nc.vector.memset(sqrt_dummy[:], 1.0)
    for c in range(NCH):
        eng = nc.vector if c == 0 else nc.gpsimd
        v = xs[c][:].bitcast(F32)
        eng.memset(v[:, 0, :], 0.0)
        eng.memset(v[:, H + 1, :], 0.0)
        eng.memset(v[:, 1 : H + 1, 0], 0.0)
        eng.memset(v[:, 1 : H + 1, W + 1], 0.0)

    # --- sync ring (Q1): ONE fused style+x0 DMA (consecutive small DMAs on
    # a drained queue each pay ~1.4us descriptor-refetch, so fusing beats
    # style-then-x0; sem ~9.9 covers both). x1-3 are gated behind x scales
    # (emitted later) so their packets don't contend with Q10's critical
    # head weight slices on the shared DMA engines (a DMA's sem posts when
    # its SLOWEST engine drains - contention = stragglers). ---
    nc.sync.dma_start(xst0s[:], x0s_d[:])

    # --- scalar ring (Q10): weight slices back-to-back, ungated. Head
    # slices sized so tap delivery beats PE consumption: conv eats 1 tap per
    # ~1.7us at half clock from ~11.4, 1/us after the HAM grant (~conv+4.3).
    # The scalar engine capacity-blocks when >4 DMAs are outstanding on the
    # ring, so it carries ONLY these issues (squares live on DVE). ---
    nc.scalar.dma_start(wt[0][:, 0:1], wt_d[:, 0, 0:1])
    nc.scalar.dma_start(wt[0][:, 1:3], wt_d[:, 0, 1:3])
    nc.scalar.dma_start(wt[0][:, 3:5], wt_d[:, 0, 3:5])
    nc.scalar.dma_start(wt[0][:, 5:7], wt_d[:, 0, 5:7])
    nc.scalar.dma_start(wt[0][:, 7:9], wt_d[:, 0, 7:9])
    for c in range(1, NCH):
        nc.scalar.dma_start(wt[c][:, 0:3], wt_d[:, c, 0:3])
        nc.scalar.dma_start(wt[c][:, 3:6], wt_d[:, c, 3:6])
        nc.scalar.dma_start(wt[c][:, 6:9], wt_d[:, c, 6:9])
    # prefetch the Sqrt ACT table (1.3us load) off the critical path
    nc.scalar.activation(sqrt_dummy[:], sqrt_dummy[:], AF.Sqrt)

    # --- sigma squares on SCALAR (ACT Square; they queue behind the
    # capacity-paced dma issues and run ~34us on - fine, sigma is consumed
    # at wave-B g5). NOT on DVE (the tile scheduler interleaves weight-gated
    # squares ahead of the critical style/x-scale chain) and NOT on gpsimd
    # (gpsimd tensor ops are far slower and the load triggers HAM
    # duty-cycling). sqpool's 3-buf rotation paces against the DVE sums. ---
    w2tiles = {}

    def sq(c, t):
        w2 = sqpool.tile([128, COUT], BF16, tag=f"w2_{t % 3}", name="w2")
        nc.scalar.activation(w2[:], wt[c][:, t], AF.Square)
        w2tiles[(c, t)] = w2

    # --- PE: a couple of tiny pipeline-priming matmuls, then conv-sized
    # RANDOM-data primers from ~7.2us so the HAM full-clock grant
    # (first-high-toggle + ~4.3us) lands by the conv stream's start.
    # (Random f32 bits can be NaN/Inf - harmless: warm_ps is a sink row
    # that sig_mm later resets with start=True.) ---
    for _ in range(2):
        nc.tensor.matmul(warm_ps, ones_r[:], warm_src[:], start=True, stop=True)
    for _ in range(WARM_BIG):
        nc.tensor.matmul(
            sigbank[:, :], warm_lhs[:], warm_rhs[:], start=True, stop=True
        )

    # --- style scales (DVE; straight off the fused tile's first 4 cols) ---
    stv = xst0s[:, 0:NCH]
    nc.vector.tensor_scalar_mul(st_rc[:], stv, RC)
    nc.vector.tensor_mul(st2[:], stv, stv)

    # --- x modulation (DVE); x0 scaled in halves so conv's first taps can
    # start right after the fused DMA lands ---
    x0v = xst0s[:, NCH:].rearrange("p (h w) -> p h w", h=H)
    nc.vector.tensor_scalar_mul(
        xs[0][:, 1:17, 1 : W + 1], x0v[:, 0:16, :], st_rc[:, 0:1]
    )
    nc.vector.tensor_scalar_mul(
        xs[0][:, 17 : H + 1, 1 : W + 1], x0v[:, 16:32, :], st_rc[:, 0:1]
    )
    # gate: this copy READS scaled xs[0] and WRITES a corner of xst123's
    # chunk-1 region, so the x1 DMA below (WAR) can't start its packets
    # until x0 is scaled - keeping Q1 quiet during Q10's critical head
    # slices. x2/x3 are likewise gated behind xs1's scale.
    nc.vector.tensor_copy(xst123[0:1, 0, 0:1], xs[0][0:1, 1, 1:2].bitcast(F32))
    nc.sync.dma_start(xst123[:, 0], x123_d[0])
    nc.vector.tensor_scalar_mul(
        xs[1][:, 1 : H + 1, 1 : W + 1],
        xst123[:, 0].rearrange("p (h w) -> p h w", h=H),
        st_rc[:, 1:2],
    )
    nc.vector.tensor_copy(xst123[0:1, 1, 0:1], xs[1][0:1, 1, 1:2].bitcast(F32))
    nc.sync.dma_start(xst123[:, 1], x123_d[1])
    nc.sync.dma_start(xst123[:, 2], x123_d[2])
    for c in range(2, NCH):
        nc.vector.tensor_scalar_mul(
            xs[c][:, 1 : H + 1, 1 : W + 1],
            xst123[:, c - 1].rearrange("p (h w) -> p h w", h=H),
            st_rc[:, c : c + 1],
        )

    # --- scalar squares + DVE tap-sums for sigma, interleaved so the 3-buf
    # w2 rotation never overwrites an unconsumed square ---
    w2s = {}

    def sq_adds(c):
        acc = sqpool.tile([128, COUT], BF16, tag=f"w2s{c}", name="w2s")
        sq(c, 0)
        sq(c, 1)
        nc.vector.tensor_add(acc[:], w2tiles[(c, 0)][:], w2tiles[(c, 1)][:])
        for t in range(2, TAPS):
            sq(c, t)
            nc.vector.tensor_add(acc[:], acc[:], w2tiles[(c, t)][:])
        w2s[c] = acc

    for c in range(NCH):
        sq_adds(c)

    # wave A groups: (oc, h=0) -> banks pc0-3; wave B: g4-6 fresh banks, g7
    # reuses pc0 (freed by wave A's flush long before g7 starts)
    wave_a = [(oc, 0) for oc in range(NCH)]
    wave_b = [(oc, 1) for oc in range(NCH)]
    pc = {
        g: psum.tile([128, 512], F32, tag=f"pc{i}", name=f"pc{i}")
        for i, g in enumerate(wave_a)
    }

    def conv_mm(g, c, t, start, stop):
        oc, h = g
        dy, dx = t // 3, t % 3
        h0 = h * 16
        nc.tensor.matmul(
            pc[g][:],
            wt[c][:, t, oc * 128 : (oc + 1) * 128],
            xs[c][:, dy + h0 : dy + h0 + 16, dx : dx + W],
            start=start,
            stop=stop,
        )

    def sig_mm(c):
        nc.tensor.matmul(
            sig_ps, st2[:, c : c + 1], w2s[c][:], start=(c == 0), stop=(c == NCH - 1)
        )

    def sig_finalize():
        # sqrt(RC^2*q + eps) [1,512] -> PE-transpose -> [128,4] -> reciprocal
        # (an SBUF->SBUF DMA reshape has the wrong element order: the natural
        # [1,512]->[128,4] mapping is partition-major, the flush needs
        # column-major)
        nc.scalar.activation(
            sig_sq[:], sig_ps, AF.Sqrt, bias=eps_b[:], scale=RC * RC
        )
        for oc in range(NCH):
            nc.tensor.transpose(
                sigbank[:, oc : oc + 1],
                sig_sq[0:1, oc * 128 : (oc + 1) * 128],
                ones_t[:],
            )
        nc.vector.tensor_copy(sig_sd[:], sigbank[:, 0:NCH])
        nc.vector.reciprocal(sig_t[:], sig_sd[:])

    def flush(g):
        # PSUM -> SBUF scaled by sigma_inv, on DVE; waits on sig_t which lands
        # mid-wave-B - only g7's bank reuse (~26us later) depends on it.
        oc, h = g
        nc.vector.tensor_scalar_mul(
            ob_h[h][:, oc * 512 : (oc + 1) * 512], pc[g][:], sig_t[:, oc : oc + 1]
        )

    def out_dma_pair(h, pair):
        nc.sync.dma_start(
            out_d[h, pair, :, :],
            ob_h[h][:, pair * 1024 : (pair + 1) * 1024].rearrange(
                "p (a q) -> p a q", a=2
            ),
        )

    def out_dma_oc(h, oc):
        nc.sync.dma_start(
            out_d[h, oc // 2, :, oc % 2],
            ob_h[h][:, oc * 512 : (oc + 1) * 512],
        )

    # --- wave A (h=0): tap-major, group-minor ---
    for c in range(NCH):
        for t in range(TAPS):
            for g in wave_a:
                conv_mm(g, c, t, c == 0 and t == 0, c == NCH - 1 and t == TAPS - 1)

    # --- wave B (h=1): per-group serial chains; sigma reduction at g5 chunk
    # boundaries (the scalar squares only finish ~34us on, after the
    # capacity-paced dma issues), finalize after g5. All flush emissions
    # come after sig_finalize so sig_t's write precedes every flush in DVE
    # program order. ---
    for gi, g in enumerate(wave_b):
        tag = "pc0" if gi == 3 else f"pcB{gi}"
        pc[g] = psum.tile([128, 512], F32, tag=tag, name=f"pcb{gi}")
        k = 0
        for c in range(NCH):
            if gi == 1:
                sig_mm(c)
            for t in range(TAPS):
                conv_mm(g, c, t, k == 0, k == TAPS * NCH - 1)
                k += 1
        if gi == 1:
            sig_finalize()
            for ga in wave_a:
                flush(ga)
            out_dma_pair(0, 0)
            out_dma_pair(0, 1)
            flush(wave_b[0])
            flush(g)
        elif gi == 2:
            flush(g)
            out_dma_pair(1, 0)
            out_dma_oc(1, 2)
        elif gi == 3:
            flush(g)
            out_dma_oc(1, 3)


_CACHE = None


def _get_compiled():
    global _CACHE
    if _CACHE is None:
        nc = bacc.Bacc(
            "TRN2", target_bir_lowering=False, debug=False, num_devices=B
        )
        # x0s: cols 0:4 = host-transposed style [128,4], cols 4: = x chunk 0
        x0s_d = nc.dram_tensor(
            "x0s", [128, NCH + PIX], F32, kind="ExternalInput"
        ).ap()
        x123_d = nc.dram_tensor(
            "x123", [NCH - 1, 128, PIX], F32, kind="ExternalInput"
        ).ap()
        wt_d = nc.dram_tensor(
            "wt", [128, NCH, TAPS, COUT], F32R, kind="ExternalInput"
        ).ap()
        # out layout: [h, oc_pair, 128, 2, 512] -> host reassembles
        out_d = nc.dram_tensor(
            "out", [2, 2, 128, 2, 512], F32, kind="ExternalOutput"
        ).ap()
        with tile.TileContext(nc) as tc, ExitStack() as ctx:
            _body(ctx, tc, x0s_d, x123_d, wt_d, out_d)
        nc.compile()
        _CACHE = nc
    return _CACHE


def kernel(x, style, weight):
    """x: (8,512,32,32) f32, style: (8,512) f32, weight: (512,512,3,3) f32
    -> (8,512,32,32) f32"""
    global LAST_RESULTS
    x = np.ascontiguousarray(np.asarray(x, dtype=np.float32))
    style = np.asarray(style, dtype=np.float32)
    weight = np.asarray(weight, dtype=np.float32)

    # Host-side layout only (no arithmetic): lhsT weight layout
    # wt[i_lo, c, t, o] = weight[o, c*128 + i_lo, t//3, t%3]
    wt = np.ascontiguousarray(
        weight.reshape(COUT, NCH, 128, TAPS).transpose(2, 1, 3, 0)
    )
    in_maps = []
    for b in range(B):
        xb = x[b].reshape(NCH, 128, PIX)
        # fused tile: transposed style [128,4] (st[p,c]=style[c*128+p],
        # a pure permutation) next to x chunk 0
        x0s = np.ascontiguousarray(
            np.concatenate([style[b].reshape(NCH, 128).T, xb[0]], axis=1)
        )
        in_maps.append(
            {
                "x0s": x0s,
                "x123": np.ascontiguousarray(xb[1:]),
                "wt": wt,
            }
        )

    nc = _get_compiled()
    res = run_bass_kernel_spmd(nc, in_maps, list(range(B)), trace=TRACE)
    LAST_RESULTS = res
    out = np.empty((B, COUT, H, W), dtype=np.float32)
    for b in range(B):
        # out HBM [h, pair, 128(cout_lo), j(oc in pair), 512(pix half)]
        o = res.results[b]["out"]
        o = o.transpose(1, 3, 2, 0, 4)  # -> [pair, j, cout_lo, h, q]
        out[b] = o.reshape(COUT, H, W)
    return out
